# revision 13
# baseline (speedup 1.0000x reference)
"""Trainium2 Bass kernel for the BANLayer problem.

Computation (per batch):
    Uc   = relu(h_c @ U_w.T + U_b)            # (N, D)
    Vp   = relu(h_p @ V_w.T + V_b)            # (M, D)
    attn = Uc @ Vp.T                          # (N, M)
    w    = softmax(attn, axis=-1)
    ctx  = w @ Vp                             # (N, D)
    out  = mean_n((Uc + ctx) * q)             # (D,)

Device algorithm (data-parallel over batch, 8 cores x 8 batches):
    mean-factorized:  out = (q/N) * (sum_n Uc[n,:] + sum_m c[m] * Vp[m,:])
    with  c[m] = sum_n E[n,m] / s[n],  E = exp(attn - 40),  s[n] = sum_m E[n,m].
    The constant shift is exact for softmax (shift invariance).

Engine balance. Only ACT and DVE may touch PSUM, and per batch the
PSUM-side elementwise work (4 exp chunks, 3 relu evictions, 2 ctx dump
halves) adds to ~8.5us -- just above 2x the 4.05us PE matmul window.
So it is split almost exactly in half:
    ACT : exp on chunks 1-3 with fused accumulator row-sums, plus the
          first 320 columns of chunk 0                           ~4.3us
    DVE : Schraudolph bit-exp (tensor_scalar fp32->int32, bitcast
          read as f32) on the last 704 columns of chunk 0, all three
          relu evictions (the ucT one carries a fused add-accumulator
          giving sum_n Uc for free), both ctx dump halves, one
          reciprocal                                             ~4.3us
    Pool: SBUF-only work: row-sum of the Schraudolph piece (via the
          tensor_scalar fused accumulator), 1/s divides for the two
          late chunks, final combines                            ~2.0us
    SP  : all HBM DMA dispatches

The Schraudolph exp has ~3% elementwise sawtooth error that cancels
through the softmax normalization: end-to-end output error ~3e-4
(gate is 2e-2). The c-pass matmuls use a stride-0 broadcast AP of
r = 1/s as the stationary operand. The c accumulation group is closed
per PSUM bank half so each dump half can start as soon as its bank is
complete, letting the batch pipeline reuse the single cb PSUM buffer.
"""

import math
import sys

import numpy as np

sys.path.insert(0, "/opt/trn_rl_repo")

B, N, M, D = 64, 512, 1024, 128
CORES = 8
BL = B // CORES  # local batches per core
SHIFT = 40.0  # softmax logit shift (exact by shift invariance)
NCH = N // 128  # n-chunks per batch (4)
MH = M // 512   # m-halves (2)
XS = 512        # chunk-0 columns handled by ACT; rest by DVE bit-exp

# Schraudolph fast-exp constants: exp(x) ~= bits(int32(x*A + C)) as f32.
# C folds in: the fp32 exponent bias, the mean-error-minimizing offset
# 366393, the -SHIFT logit shift, and +0.5 to counter the truncating
# float->int32 conversion.
A_EXP = float(2 ** 7 / math.log(2))
C_EXP = float(127 * 2 ** 7 - 7.4 - A_EXP * SHIFT + 0.5)

_BUILt = {}


def _build_nc():
    import concourse.bass as bass  # noqa: F401
    import concourse.tile as tile
    from concourse import bacc, mybir

    F32 = mybir.dt.float32
    F32R = mybir.dt.float32r
    BF16 = mybir.dt.bfloat16
    I16 = mybir.dt.int16
    OP = mybir.AluOpType
    ACTF = mybir.ActivationFunctionType

    nc = bacc.Bacc("TRN2", target_bir_lowering=False, debug=False,
                   num_devices=CORES)

    hcT = nc.declare_dram_parameter("hcT", [BL, D, N], F32, isOutput=False)
    hpT = nc.declare_dram_parameter("hpT", [BL, D, M], F32, isOutput=False)
    w2 = nc.declare_dram_parameter("w2", [D, 2 * D], F32, isOutput=False)
    bias3 = nc.declare_dram_parameter("bias3", [D, 3], F32, isOutput=False)
    y = nc.declare_dram_parameter("y", [D, BL], F32, isOutput=True)

    with tile.TileContext(nc) as tc:
        with (
            tc.tile_pool(name="consts", bufs=1) as consts,
            tc.tile_pool(name="inp", bufs=3) as inp,
            tc.tile_pool(name="uct", bufs=3) as uctp,
            tc.tile_pool(name="vpt", bufs=3) as vptp,
            tc.tile_pool(name="epool", bufs=10) as epool,
            tc.tile_pool(name="scratch", bufs=4) as scratch,
            tc.tile_pool(name="stats", bufs=3) as stats,
            tc.tile_pool(name="psA", bufs=2, space="PSUM") as psA,
            tc.tile_pool(name="psB", bufs=2, space="PSUM") as psB,
            tc.tile_pool(name="psC", bufs=1, space="PSUM") as psC,
        ):
            # ---- constants ----
            w2_sb = consts.tile([D, 2 * D], F32)
            nc.sync.dma_start(w2_sb[:].bitcast(F32R), w2[:].bitcast(F32R))
            uwT_sb = w2_sb[:, 0:D]
            vwT_sb = w2_sb[:, D:2 * D]
            b3_sb = consts.tile([D, 3], F32)
            ub_sb = b3_sb[:, 0:1]
            vb_sb = b3_sb[:, 1:2]
            qn_sb = b3_sb[:, 2:3]

            nshift = consts.tile([128, 1], F32)
            nc.vector.memset(nshift[:], -SHIFT)
            zeros = consts.tile([128, 512], F32)
            nc.vector.memset(zeros[:], 0.0)
            ones1 = consts.tile([128, 1], F32)
            nc.vector.memset(ones1[:], 1.0)
            y_sb = consts.tile([D, BL], F32)

            hcs, hps = [], []

            def load_batch(b):
                hc = inp.tile([D, N], F32, name="hc")
                hp = inp.tile([D, M], F32, name="hp")
                nc.sync.dma_start(hc[:].bitcast(F32R), hcT[b].bitcast(F32R))
                nc.sync.dma_start(hp[:].bitcast(F32R), hpT[b].bitcast(F32R))
                hcs.append(hc)
                hps.append(hp)

            load_batch(0)
            nc.sync.dma_start(b3_sb[:], bias3[:])
            load_batch(1)

            # PE p-state warmup while hc/hp stream in (psA: free at start).
            wu_ps = psA.tile([128, 256], F32, name="wu_ps", tag="att")
            for _ in range(5):
                nc.tensor.matmul(wu_ps[:, 0:256], w2_sb[:, 0:1]
                                 .broadcast_to([128, 128]).bitcast(F32R),
                                 w2_sb[:].bitcast(F32R),
                                 start=True, stop=True)

            # per-batch pipeline state
            ucTs = [None] * BL
            vpTs = [None] * BL
            es = [None] * BL
            s4s = [None] * BL   # [128, 4] row-sums (chunk 0 assembled)
            s0xs = [None] * BL  # chunk-0 partial sum from the bit-exp piece
            e0xs = [None] * BL  # chunk-0 bit-exp tile (int32 bits in f32)
            r4s = [None] * BL
            rb0s = [None] * BL  # bf16 copy of r[0] for the bit-exp cb MMs
            ucsums = [None] * BL
            cbs = [None] * BL
            yctxs = [None] * BL

            def proj_uc(b):
                uc_ps = psB.tile([128, N], F32, name="uc_ps", tag="proj",
                                 bufs=1)
                nc.tensor.matmul(uc_ps[:], uwT_sb.bitcast(F32R),
                                 hcs[b][:].bitcast(F32R),
                                 start=True, stop=True)
                return uc_ps

            def evict_uc(b, uc_ps):
                # DVE relu eviction with fused add-accumulator:
                # ucsum = sum_n Uc[n, :] comes for free.
                ucT = uctp.tile([D, N], F32, name="ucT")
                ucsum = stats.tile([D, 1], F32, name="ucsum")
                nc.vector.scalar_tensor_tensor(ucT[:].bitcast(F32R), uc_ps[:], ub_sb,
                                               zeros[:], OP.add, OP.max,
                                               accum_out=ucsum[:])
                ucTs[b] = ucT
                ucsums[b] = ucsum

            def proj_vp(b, h, _=None):
                if vpTs[b] is None:
                    vpTs[b] = vptp.tile([D, M], F32, name="vpT")
                vp_ps = psB.tile([128, 512], F32, name="vp_ps", tag="proj",
                                 bufs=1)
                nc.tensor.matmul(vp_ps[:], vwT_sb.bitcast(F32R),
                                 hps[b][:, h * 512:(h + 1) * 512]
                                 .bitcast(F32R), start=True, stop=True)
                return vp_ps

            def evict_vp_h(b, h, vp_ps):
                nc.vector.tensor_scalar(
                    vpTs[b][:, h * 512:(h + 1) * 512].bitcast(F32R),
                    vp_ps[:], vb_sb, 0.0, OP.add, OP.max)

            def att_mm(b, j):
                att_ps = psA.tile([128, 1024], F32, name="att_ps", tag="att")
                lhs = ucTs[b][:, j * 128:(j + 1) * 128]
                for h in range(MH):
                    nc.tensor.matmul(att_ps[:, h * 512:(h + 1) * 512],
                                     lhs.bitcast(F32R),
                                     vpTs[b][:, h * 512:(h + 1) * 512]
                                     .bitcast(F32R), start=True, stop=True)
                return att_ps

            def exp_act(b, j, att_ps):
                # full-chunk table exp on ACT with fused row-sum accumulator
                e_sb = epool.tile([128, M], F32, name="e_sb")
                nc.scalar.activation(e_sb[:].bitcast(F32R), att_ps[:],
                                     ACTF.Exp, bias=nshift[:], scale=1.0,
                                     accum_out=s4s[b][:, j:j + 1])
                es[b][j] = e_sb

            def exp_act_part(b, att_ps):
                # ACT slice of chunk 0: first XS columns (+ row-sum part).
                # Separate tile from the bit-exp piece: the FP32r verifier
                # tracks rounded-ness per memory location.
                es[b][0] = epool.tile([128, XS], F32, name="e0a")
                s0a = stats.tile([128, 1], F32, name="s0a")
                nc.scalar.activation(es[b][0][:].bitcast(F32R),
                                     att_ps[:, 0:XS], ACTF.Exp,
                                     bias=nshift[:], scale=1.0,
                                     accum_out=s0a[:])
                return s0a

            def exp_dve_part(b, att_ps):
                # DVE Schraudolph bit-exp on chunk-0 columns [XS:1024]:
                # int16(att*A + C) bits ARE the bf16 of exp(att - SHIFT).
                e0x = epool.tile([128, M - XS], I16, name="e0x")
                nc.vector.tensor_scalar(
                    e0x[:], att_ps[:, XS:M],
                    A_EXP, C_EXP, OP.mult, OP.add)
                e0xs[b] = e0x

            def s0_sum_dve(b):
                # DVE row-sum of the bit-exp piece (2-byte packed -> 2x)
                s0x = stats.tile([128, 1], F32, name="s0x")
                nc.vector.tensor_reduce(s0x[:], e0xs[b][:].bitcast(BF16),
                                        mybir.AxisListType.X, OP.add)
                s0xs[b] = s0x
                return s0x

            def s0_add_pool(b, s0a):
                eng = nc.vector if b < 2 else nc.gpsimd
                eng.tensor_tensor(s4s[b][:, 0:1], s0a[:],
                                  s0xs[b][:], OP.add)

            def cb_mm(b, j, h, start, stop):
                if cbs[b] is None:
                    cbs[b] = psC.tile([128, M], F32, name="cb_ps")
                if j == 0:
                    # chunk 0 split at the bank boundary: [0:512] from ACT
                    # (f32r), [512:1024] is int16 bit-exp = packed bf16
                    if h == 0:
                        lhs = r4s[b][:, 0:1].broadcast_to([128, 128]) \
                            .bitcast(F32R)
                        nc.tensor.matmul(
                            cbs[b][:, 0:XS], lhs,
                            es[b][0][:].bitcast(F32R),
                            start=start, stop=stop)
                    else:
                        nc.tensor.matmul(
                            cbs[b][:, XS:M],
                            rb0s[b][:].broadcast_to([128, 128]),
                            e0xs[b][:].bitcast(BF16),
                            start=start, stop=stop)
                    return
                rhs = es[b][j][:, h * 512:(h + 1) * 512].bitcast(F32R)
                lhs = r4s[b][:, j:j + 1].broadcast_to([128, 128]) \
                    .bitcast(F32R)
                nc.tensor.matmul(cbs[b][:, h * 512:(h + 1) * 512], lhs, rhs,
                                 start=start, stop=stop)

            def dump_half(b, h):
                # DVE: yctx_h[e] = sum_{m in half} Vp^T[e,m] * c[m]
                if yctxs[b] is None:
                    yctxs[b] = stats.tile([D, 2], F32, name="yctx")
                dmp = scratch.tile([128, 512], F32, name="dump")
                nc.vector.scalar_tensor_tensor(
                    dmp[:], vpTs[b][:, h * 512:(h + 1) * 512], 1.0,
                    cbs[b][:, h * 512:(h + 1) * 512], OP.mult, OP.mult,
                    accum_out=yctxs[b][:, h:h + 1])

            def combines(b):
                t1 = stats.tile([D, 1], F32, name="t1")
                nc.gpsimd.tensor_tensor(t1[:], ucsums[b][:],
                                        yctxs[b][:, 0:1], OP.add)
                t2 = stats.tile([D, 1], F32, name="t2")
                nc.gpsimd.tensor_tensor(t2[:], t1[:], yctxs[b][:, 1:2],
                                        OP.add)
                nc.gpsimd.tensor_scalar(y_sb[:, b:b + 1], t2[:], qn_sb,
                                        None, OP.mult)

            def rb0_pool(b):
                # bf16 copy of r0 for the bit-exp cb matmul (Pool mult)
                rb0 = stats.tile([128, 1], BF16, name="rb0")
                nc.gpsimd.tensor_scalar(rb0[:], ones1[:], r4s[b][:, 0:1],
                                        None, OP.mult)
                rb0s[b] = rb0

            # ---- prologue: batch 0 projections ----
            uc_ps0 = proj_uc(0)
            evict_uc(0, uc_ps0)
            vp0 = proj_vp(0, 0)
            evict_vp_h(0, 0, vp0)
            vp0b = proj_vp(0, 1)
            evict_vp_h(0, 1, vp0b)

            for b in range(BL):
                s4s[b] = stats.tile([128, NCH], F32, name="s4")
                r4s[b] = stats.tile([128, NCH], F32, name="r4")
                es[b] = [None] * NCH
                p = b - 1

                att0 = att_mm(b, 0)
                s0a = exp_act_part(b, att0)      # ACT
                exp_dve_part(b, att0)            # DVE
                att1 = att_mm(b, 1)
                exp_act(b, 1, att1)              # ACT
                if p >= 0:
                    cb_mm(p, 0, 0, True, False)
                    cb_mm(p, 1, 0, False, False)
                    cb_mm(p, 2, 0, False, False)
                s0_sum_dve(b)                    # DVE
                s0_add_pool(b, s0a)              # Pool
                if b + 1 < BL:
                    uc_ps = proj_uc(b + 1)
                    evict_uc(b + 1, uc_ps)       # DVE
                att2 = att_mm(b, 2)
                exp_act(b, 2, att2)              # ACT
                with nc.allow_low_precision(reason="r is a matmul operand"):
                    nc.vector.reciprocal(r4s[b][:, 0:2].bitcast(F32R),
                                         s4s[b][:, 0:2])  # DVE
                rb0_pool(b)                      # Pool
                att3 = att_mm(b, 3)
                exp_act(b, 3, att3)              # ACT
                if p >= 0:
                    cb_mm(p, 0, 1, True, False)
                    cb_mm(p, 1, 1, False, False)
                if b + 1 < BL:
                    vp = proj_vp(b + 1, 0)
                    evict_vp_h(b + 1, 0, vp)     # DVE
                    vpb_ = proj_vp(b + 1, 1)
                    evict_vp_h(b + 1, 1, vpb_)   # DVE
                if p >= 0:
                    cb_mm(p, 2, 1, False, False)
                    cb_mm(p, 3, 0, False, True)   # closes bank a
                    dump_half(p, 0)               # DVE
                    cb_mm(p, 3, 1, False, True)   # closes bank b
                    dump_half(p, 1)               # DVE
                    combines(p)                   # Pool
                with nc.allow_low_precision(reason="r is a matmul operand"):
                    nc.vector.reciprocal(r4s[b][:, 2:4].bitcast(F32R),
                                         s4s[b][:, 2:4])  # DVE
                if b + 2 < BL:
                    load_batch(b + 2)

            # ---- epilogue: last batch's cb group, dumps, combine ----
            L = BL - 1
            cb_mm(L, 0, 0, True, False)
            cb_mm(L, 0, 1, True, False)
            cb_mm(L, 1, 0, False, False)
            cb_mm(L, 1, 1, False, False)
            cb_mm(L, 2, 0, False, False)
            cb_mm(L, 2, 1, False, False)
            cb_mm(L, 3, 0, False, True)
            dump_half(L, 0)
            cb_mm(L, 3, 1, False, True)
            dump_half(L, 1)
            combines(L)
            nc.sync.dma_start(y[:], y_sb[:])

    nc.finalize()
    return nc


def kernel(h_c, h_p, U_w, U_b, V_w, V_b, q):
    from concourse.bass_utils import run_bass_kernel_spmd

    if "nc" not in _BUILt:
        _BUILt["nc"] = _build_nc()
    nc = _BUILt["nc"]

    h_c = np.ascontiguousarray(np.asarray(h_c, dtype=np.float32))
    h_p = np.ascontiguousarray(np.asarray(h_p, dtype=np.float32))
    w2 = np.ascontiguousarray(np.concatenate(
        [np.asarray(U_w, dtype=np.float32).T,
         np.asarray(V_w, dtype=np.float32).T], axis=1))
    bias3 = np.ascontiguousarray(np.stack(
        [np.asarray(U_b, dtype=np.float32),
         np.asarray(V_b, dtype=np.float32),
         np.asarray(q, dtype=np.float32) / np.float32(N)], axis=1))

    in_maps = []
    for c in range(CORES):
        sl = slice(c * BL, (c + 1) * BL)
        in_maps.append({
            "hcT": np.ascontiguousarray(h_c[sl].transpose(0, 2, 1)),
            "hpT": np.ascontiguousarray(h_p[sl].transpose(0, 2, 1)),
            "w2": w2, "bias3": bias3,
        })

    global _last_in_maps
    _last_in_maps = in_maps
    res = run_bass_kernel_spmd(nc, in_maps, core_ids=list(range(CORES)))
    out = np.empty((B, D), dtype=np.float32)
    for c in range(CORES):
        out[c * BL:(c + 1) * BL] = res.results[c]["y"].T
    return out


# revision 14
# speedup vs baseline: 1.0413x; 1.0413x over previous
"""Trainium2 Bass kernel for the BANLayer problem.

Computation (per batch):
    Uc   = relu(h_c @ U_w.T + U_b)            # (N, D)
    Vp   = relu(h_p @ V_w.T + V_b)            # (M, D)
    attn = Uc @ Vp.T                          # (N, M)
    w    = softmax(attn, axis=-1)
    ctx  = w @ Vp                             # (N, D)
    out  = mean_n((Uc + ctx) * q)             # (D,)

Device algorithm (data-parallel over batch, 8 cores x 8 batches):
    mean-factorized:  out = (q/N) * (sum_n Uc[n,:] + sum_m c[m] * Vp[m,:])
    with  c[m] = sum_n E[n,m] / s[n],  E = exp(attn - 40),  s[n] = sum_m E[n,m].
    The constant shift is exact for softmax (shift invariance).

Engine balance. Only ACT and DVE may touch PSUM, and per batch the
PSUM-side elementwise work (4 exp chunks, 3 relu evictions, 2 ctx dump
halves) adds to ~8.5us -- just above 2x the 4.05us PE matmul window.
So it is split almost exactly in half:
    ACT : exp on chunks 1-3 with fused accumulator row-sums, plus the
          first 320 columns of chunk 0                           ~4.3us
    DVE : Schraudolph bit-exp (tensor_scalar fp32->int32, bitcast
          read as f32) on the last 704 columns of chunk 0, all three
          relu evictions (the ucT one carries a fused add-accumulator
          giving sum_n Uc for free), both ctx dump halves, one
          reciprocal                                             ~4.3us
    Pool: SBUF-only work: row-sum of the Schraudolph piece (via the
          tensor_scalar fused accumulator), 1/s divides for the two
          late chunks, final combines                            ~2.0us
    SP  : all HBM DMA dispatches

The Schraudolph exp has ~3% elementwise sawtooth error that cancels
through the softmax normalization: end-to-end output error ~3e-4
(gate is 2e-2). The c-pass matmuls use a stride-0 broadcast AP of
r = 1/s as the stationary operand. The c accumulation group is closed
per PSUM bank half so each dump half can start as soon as its bank is
complete, letting the batch pipeline reuse the single cb PSUM buffer.
"""

import math
import sys

import numpy as np

sys.path.insert(0, "/opt/trn_rl_repo")

B, N, M, D = 64, 512, 1024, 128
CORES = 8
BL = B // CORES  # local batches per core
SHIFT = 40.0  # softmax logit shift (exact by shift invariance)
NCH = N // 128  # n-chunks per batch (4)
MH = M // 512   # m-halves (2)
XS = 512        # chunk-0 columns handled by ACT; rest by DVE bit-exp

# Schraudolph fast-exp constants: exp(x) ~= bits(int32(x*A + C)) as f32.
# C folds in: the fp32 exponent bias, the mean-error-minimizing offset
# 366393, the -SHIFT logit shift, and +0.5 to counter the truncating
# float->int32 conversion.
A_EXP = float(2 ** 7 / math.log(2))
C_EXP = float(127 * 2 ** 7 - 7.4 - A_EXP * SHIFT + 0.5)

_BUILt = {}


def _build_nc():
    import concourse.bass as bass  # noqa: F401
    import concourse.tile as tile
    from concourse import bacc, mybir

    F32 = mybir.dt.float32
    F32R = mybir.dt.float32r
    BF16 = mybir.dt.bfloat16
    I16 = mybir.dt.int16
    OP = mybir.AluOpType
    ACTF = mybir.ActivationFunctionType

    nc = bacc.Bacc("TRN2", target_bir_lowering=False, debug=False,
                   num_devices=CORES)

    hcT = nc.declare_dram_parameter("hcT", [BL, D, N], F32, isOutput=False)
    hpT = nc.declare_dram_parameter("hpT", [BL, D, M], F32, isOutput=False)
    w2 = nc.declare_dram_parameter("w2", [D, 2 * D], F32, isOutput=False)
    bias3 = nc.declare_dram_parameter("bias3", [D, 3], F32, isOutput=False)
    y = nc.declare_dram_parameter("y", [D, BL], F32, isOutput=True)

    with tile.TileContext(nc) as tc:
        with (
            tc.tile_pool(name="consts", bufs=1) as consts,
            tc.tile_pool(name="inp", bufs=3) as inp,
            tc.tile_pool(name="uct", bufs=3) as uctp,
            tc.tile_pool(name="vpt", bufs=3) as vptp,
            tc.tile_pool(name="epool", bufs=10) as epool,
            tc.tile_pool(name="scratch", bufs=4) as scratch,
            tc.tile_pool(name="stats", bufs=3) as stats,
            tc.tile_pool(name="psA", bufs=2, space="PSUM") as psA,
            tc.tile_pool(name="psB", bufs=2, space="PSUM") as psB,
            tc.tile_pool(name="psC", bufs=1, space="PSUM") as psC,
        ):
            # ---- constants ----
            w2_sb = consts.tile([D, 2 * D], F32)
            nc.sync.dma_start(w2_sb[:].bitcast(F32R), w2[:].bitcast(F32R))
            uwT_sb = w2_sb[:, 0:D]
            vwT_sb = w2_sb[:, D:2 * D]
            b3_sb = consts.tile([D, 3], F32)
            ub_sb = b3_sb[:, 0:1]
            vb_sb = b3_sb[:, 1:2]
            qn_sb = b3_sb[:, 2:3]

            nshift = consts.tile([128, 1], F32)
            nc.vector.memset(nshift[:], -SHIFT)
            zeros = consts.tile([128, 512], F32)
            nc.vector.memset(zeros[:], 0.0)
            ones1 = consts.tile([128, 1], F32)
            nc.vector.memset(ones1[:], 1.0)
            y_sb = consts.tile([D, BL], F32)

            hcs, hps = [], []

            def load_batch(b):
                hc = inp.tile([D, N], F32, name="hc")
                hp = inp.tile([D, M], F32, name="hp")
                nc.sync.dma_start(hc[:].bitcast(F32R), hcT[b].bitcast(F32R))
                nc.sync.dma_start(hp[:].bitcast(F32R), hpT[b].bitcast(F32R))
                hcs.append(hc)
                hps.append(hp)

            load_batch(0)
            nc.sync.dma_start(b3_sb[:], bias3[:])
            load_batch(1)

            # PE p-state warmup while hc/hp stream in (psA: free at start).
            wu_ps = psA.tile([128, 256], F32, name="wu_ps", tag="att")
            for _ in range(5):
                nc.tensor.matmul(wu_ps[:, 0:256], w2_sb[:, 0:1]
                                 .broadcast_to([128, 128]).bitcast(F32R),
                                 w2_sb[:].bitcast(F32R),
                                 start=True, stop=True)

            # per-batch pipeline state
            ucTs = [None] * BL
            vpTs = [None] * BL
            es = [None] * BL
            s4s = [None] * BL   # [128, 4] row-sums (chunk 0 assembled)
            s0xs = [None] * BL  # chunk-0 partial sum from the bit-exp piece
            e0xs = [None] * BL  # chunk-0 bit-exp tile (int32 bits in f32)
            r4s = [None] * BL
            rb0s = [None] * BL  # bf16 copy of r[0] for the bit-exp cb MMs
            ucsums = [None] * BL
            cbs = [None] * BL
            yctxs = [None] * BL

            def proj_uc(b):
                uc_ps = psB.tile([128, N], F32, name="uc_ps", tag="proj",
                                 bufs=2)
                nc.tensor.matmul(uc_ps[:], uwT_sb.bitcast(F32R),
                                 hcs[b][:].bitcast(F32R),
                                 start=True, stop=True)
                return uc_ps

            def evict_uc(b, uc_ps):
                # DVE relu eviction with fused add-accumulator:
                # ucsum = sum_n Uc[n, :] comes for free.
                ucT = uctp.tile([D, N], F32, name="ucT")
                ucsum = stats.tile([D, 1], F32, name="ucsum")
                nc.vector.scalar_tensor_tensor(ucT[:].bitcast(F32R), uc_ps[:], ub_sb,
                                               zeros[:], OP.add, OP.max,
                                               accum_out=ucsum[:])
                ucTs[b] = ucT
                ucsums[b] = ucsum

            def proj_vp(b, h, _=None):
                if vpTs[b] is None:
                    vpTs[b] = vptp.tile([D, M], F32, name="vpT")
                vp_ps = psB.tile([128, 512], F32, name="vp_ps", tag="proj",
                                 bufs=2)
                nc.tensor.matmul(vp_ps[:], vwT_sb.bitcast(F32R),
                                 hps[b][:, h * 512:(h + 1) * 512]
                                 .bitcast(F32R), start=True, stop=True)
                return vp_ps

            def evict_vp_h(b, h, vp_ps):
                nc.vector.tensor_scalar(
                    vpTs[b][:, h * 512:(h + 1) * 512].bitcast(F32R),
                    vp_ps[:], vb_sb, 0.0, OP.add, OP.max)

            def att_mm(b, j):
                att_ps = psA.tile([128, 1024], F32, name="att_ps", tag="att")
                lhs = ucTs[b][:, j * 128:(j + 1) * 128]
                for h in range(MH):
                    nc.tensor.matmul(att_ps[:, h * 512:(h + 1) * 512],
                                     lhs.bitcast(F32R),
                                     vpTs[b][:, h * 512:(h + 1) * 512]
                                     .bitcast(F32R), start=True, stop=True)
                return att_ps

            def exp_act(b, j, att_ps):
                # full-chunk table exp on ACT with fused row-sum accumulator
                e_sb = epool.tile([128, M], F32, name="e_sb")
                nc.scalar.activation(e_sb[:].bitcast(F32R), att_ps[:],
                                     ACTF.Exp, bias=nshift[:], scale=1.0,
                                     accum_out=s4s[b][:, j:j + 1])
                es[b][j] = e_sb

            def exp_act_part(b, att_ps):
                # ACT slice of chunk 0: first XS columns (+ row-sum part).
                # Separate tile from the bit-exp piece: the FP32r verifier
                # tracks rounded-ness per memory location.
                es[b][0] = epool.tile([128, XS], F32, name="e0a")
                s0a = stats.tile([128, 1], F32, name="s0a")
                nc.scalar.activation(es[b][0][:].bitcast(F32R),
                                     att_ps[:, 0:XS], ACTF.Exp,
                                     bias=nshift[:], scale=1.0,
                                     accum_out=s0a[:])
                return s0a

            def exp_dve_part(b, att_ps):
                # DVE Schraudolph bit-exp on chunk-0 columns [XS:1024]:
                # int16(att*A + C) bits ARE the bf16 of exp(att - SHIFT).
                e0x = epool.tile([128, M - XS], I16, name="e0x")
                nc.vector.tensor_scalar(
                    e0x[:], att_ps[:, XS:M],
                    A_EXP, C_EXP, OP.mult, OP.add)
                e0xs[b] = e0x

            def s0_sum_dve(b):
                # DVE row-sum of the bit-exp piece (2-byte packed -> 2x)
                s0x = stats.tile([128, 1], F32, name="s0x")
                nc.vector.tensor_reduce(s0x[:], e0xs[b][:].bitcast(BF16),
                                        mybir.AxisListType.X, OP.add)
                s0xs[b] = s0x
                return s0x

            def s0_add_pool(b, s0a):
                eng = nc.vector if b < 2 else nc.gpsimd
                eng.tensor_tensor(s4s[b][:, 0:1], s0a[:],
                                  s0xs[b][:], OP.add)

            def cb_mm(b, j, h, start, stop):
                if cbs[b] is None:
                    cbs[b] = psC.tile([128, M], F32, name="cb_ps")
                if j == 0:
                    # chunk 0 split at the bank boundary: [0:512] from ACT
                    # (f32r), [512:1024] is int16 bit-exp = packed bf16
                    if h == 0:
                        lhs = r4s[b][:, 0:1].broadcast_to([128, 128]) \
                            .bitcast(F32R)
                        nc.tensor.matmul(
                            cbs[b][:, 0:XS], lhs,
                            es[b][0][:].bitcast(F32R),
                            start=start, stop=stop)
                    else:
                        nc.tensor.matmul(
                            cbs[b][:, XS:M],
                            rb0s[b][:].broadcast_to([128, 128]),
                            e0xs[b][:].bitcast(BF16),
                            start=start, stop=stop)
                    return
                rhs = es[b][j][:, h * 512:(h + 1) * 512].bitcast(F32R)
                lhs = r4s[b][:, j:j + 1].broadcast_to([128, 128]) \
                    .bitcast(F32R)
                nc.tensor.matmul(cbs[b][:, h * 512:(h + 1) * 512], lhs, rhs,
                                 start=start, stop=stop)

            def dump_half(b, h):
                # DVE: yctx_h[e] = sum_{m in half} Vp^T[e,m] * c[m]
                if yctxs[b] is None:
                    yctxs[b] = stats.tile([D, 2], F32, name="yctx")
                dmp = scratch.tile([128, 512], F32, name="dump")
                nc.vector.scalar_tensor_tensor(
                    dmp[:], vpTs[b][:, h * 512:(h + 1) * 512], 1.0,
                    cbs[b][:, h * 512:(h + 1) * 512], OP.mult, OP.mult,
                    accum_out=yctxs[b][:, h:h + 1])

            def combines(b):
                t1 = stats.tile([D, 1], F32, name="t1")
                nc.gpsimd.tensor_tensor(t1[:], ucsums[b][:],
                                        yctxs[b][:, 0:1], OP.add)
                t2 = stats.tile([D, 1], F32, name="t2")
                nc.gpsimd.tensor_tensor(t2[:], t1[:], yctxs[b][:, 1:2],
                                        OP.add)
                nc.gpsimd.tensor_scalar(y_sb[:, b:b + 1], t2[:], qn_sb,
                                        None, OP.mult)

            def rb0_pool(b):
                # bf16 copy of r0 for the bit-exp cb matmul (Pool mult)
                rb0 = stats.tile([128, 1], BF16, name="rb0")
                nc.gpsimd.tensor_scalar(rb0[:], ones1[:], r4s[b][:, 0:1],
                                        None, OP.mult)
                rb0s[b] = rb0

            # ---- prologue: batch 0 projections ----
            uc_ps0 = proj_uc(0)
            evict_uc(0, uc_ps0)
            vp0 = proj_vp(0, 0)
            evict_vp_h(0, 0, vp0)
            vp0b = proj_vp(0, 1)
            evict_vp_h(0, 1, vp0b)

            for b in range(BL):
                s4s[b] = stats.tile([128, NCH], F32, name="s4")
                r4s[b] = stats.tile([128, NCH], F32, name="r4")
                es[b] = [None] * NCH
                p = b - 1

                att0 = att_mm(b, 0)
                s0a = exp_act_part(b, att0)      # ACT
                exp_dve_part(b, att0)            # DVE
                att1 = att_mm(b, 1)
                exp_act(b, 1, att1)              # ACT
                if p >= 0:
                    cb_mm(p, 0, 0, True, False)
                    cb_mm(p, 1, 0, False, False)
                    cb_mm(p, 2, 0, False, False)
                s0_sum_dve(b)                    # DVE
                s0_add_pool(b, s0a)              # Pool
                if b + 1 < BL:
                    uc_ps = proj_uc(b + 1)
                    evict_uc(b + 1, uc_ps)       # DVE
                att2 = att_mm(b, 2)
                exp_act(b, 2, att2)              # ACT
                with nc.allow_low_precision(reason="r is a matmul operand"):
                    nc.vector.reciprocal(r4s[b][:, 0:2].bitcast(F32R),
                                         s4s[b][:, 0:2])  # DVE
                rb0_pool(b)                      # Pool
                att3 = att_mm(b, 3)
                exp_act(b, 3, att3)              # ACT
                if p >= 0:
                    cb_mm(p, 0, 1, True, False)
                    cb_mm(p, 1, 1, False, False)
                if b + 1 < BL:
                    vp = proj_vp(b + 1, 0)
                    evict_vp_h(b + 1, 0, vp)     # DVE
                    vpb_ = proj_vp(b + 1, 1)
                    evict_vp_h(b + 1, 1, vpb_)   # DVE
                if p >= 0:
                    cb_mm(p, 2, 1, False, False)
                    cb_mm(p, 3, 0, False, True)   # closes bank a
                    dump_half(p, 0)               # DVE
                    cb_mm(p, 3, 1, False, True)   # closes bank b
                    dump_half(p, 1)               # DVE
                    combines(p)                   # Pool
                with nc.allow_low_precision(reason="r is a matmul operand"):
                    nc.vector.reciprocal(r4s[b][:, 2:4].bitcast(F32R),
                                         s4s[b][:, 2:4])  # DVE
                if b + 2 < BL:
                    load_batch(b + 2)

            # ---- epilogue: last batch's cb group, dumps, combine ----
            L = BL - 1
            cb_mm(L, 0, 0, True, False)
            cb_mm(L, 0, 1, True, False)
            cb_mm(L, 1, 0, False, False)
            cb_mm(L, 1, 1, False, False)
            cb_mm(L, 2, 0, False, False)
            cb_mm(L, 2, 1, False, False)
            cb_mm(L, 3, 0, False, True)
            dump_half(L, 0)
            cb_mm(L, 3, 1, False, True)
            dump_half(L, 1)
            combines(L)
            nc.sync.dma_start(y[:], y_sb[:])

    nc.finalize()
    return nc


def kernel(h_c, h_p, U_w, U_b, V_w, V_b, q):
    from concourse.bass_utils import run_bass_kernel_spmd

    if "nc" not in _BUILt:
        _BUILt["nc"] = _build_nc()
    nc = _BUILt["nc"]

    h_c = np.ascontiguousarray(np.asarray(h_c, dtype=np.float32))
    h_p = np.ascontiguousarray(np.asarray(h_p, dtype=np.float32))
    w2 = np.ascontiguousarray(np.concatenate(
        [np.asarray(U_w, dtype=np.float32).T,
         np.asarray(V_w, dtype=np.float32).T], axis=1))
    bias3 = np.ascontiguousarray(np.stack(
        [np.asarray(U_b, dtype=np.float32),
         np.asarray(V_b, dtype=np.float32),
         np.asarray(q, dtype=np.float32) / np.float32(N)], axis=1))

    in_maps = []
    for c in range(CORES):
        sl = slice(c * BL, (c + 1) * BL)
        in_maps.append({
            "hcT": np.ascontiguousarray(h_c[sl].transpose(0, 2, 1)),
            "hpT": np.ascontiguousarray(h_p[sl].transpose(0, 2, 1)),
            "w2": w2, "bias3": bias3,
        })

    global _last_in_maps
    _last_in_maps = in_maps
    res = run_bass_kernel_spmd(nc, in_maps, core_ids=list(range(CORES)))
    out = np.empty((B, D), dtype=np.float32)
    for c in range(CORES):
        out[c * BL:(c + 1) * BL] = res.results[c]["y"].T
    return out
